# revision 5
# baseline (speedup 1.0000x reference)
"""Conditional (class-routed) 3x3 SAME conv, data-parallel over batch on 8 TRN2 cores.

Strategy:
  - Host: gather per-sample expert kernel/bias (kernel[classes], bias[classes]),
    zero-pad x to 66x66 and transpose to channel-major [CIN, HP, WP]; shard
    batch 4 samples/core.
  - Device (per core): for each sample, conv = sum over 9 taps of a
    [CIN=128, F_half=128]^T @ [CIN=128, spatial-window 512] matmul accumulated
    in PSUM (fp32r = full-rate fp32 matmul). Bias added on the ACT engine
    during PSUM->SBUF eviction.
  - Host: un-transpose [F, HW] -> [H, W, F] and concatenate shards.
"""

import numpy as np

_B, _H, _W, _CIN = 32, 64, 64, 128
_F, _KH, _KW = 256, 3, 3
_NCORES = 8
_BPC = _B // _NCORES          # 4 samples per core
_HP, _WP = _H + 2, _W + 2     # 66, 66 (zero-padded)
_SP = _H * _W                 # 4096 output positions
_FH = 128                     # output-channel half (PSUM partition dim)
_NFH = _F // _FH              # 2
_CHUNK = 512                  # spatial positions per PSUM bank
_NCH = _SP // _CHUNK          # 8
_ROWS = _CHUNK // _W          # 8 output rows per chunk

_nc = None


def _build_nc():
    import concourse.bacc as bacc
    import concourse.mybir as mybir
    import concourse.tile as tile

    f32 = mybir.dt.float32
    f32r = mybir.dt.float32r
    ident = mybir.ActivationFunctionType.Identity

    nc = bacc.Bacc("TRN2", target_bir_lowering=False, debug=False)
    xT = nc.dram_tensor("xT", (_BPC, _CIN, _HP, _WP), f32r, kind="ExternalInput")
    kT = nc.dram_tensor("kT", (_BPC, _CIN, _KH * _KW, _F), f32r, kind="ExternalInput")
    bT = nc.dram_tensor("bT", (_FH, _BPC * _NFH), f32, kind="ExternalInput")
    yT = nc.dram_tensor("yT", (_BPC, _NFH, _FH, _SP), f32, kind="ExternalOutput")

    with tile.TileContext(nc) as tc:
        with (
            tc.tile_pool(name="xp", bufs=2) as xp,
            tc.tile_pool(name="kp", bufs=2) as kp,
            tc.tile_pool(name="bp", bufs=1) as bp,
            tc.tile_pool(name="op", bufs=4) as op,
            tc.tile_pool(name="ps", bufs=8, space="PSUM") as ps,
        ):
            b_sb = bp.tile([_FH, _BPC * _NFH], f32)
            nc.sync.dma_start(b_sb[:], bT[:])
            for s in range(_BPC):
                x_sb = xp.tile([_CIN, _HP, _WP], f32r)
                nc.sync.dma_start(x_sb[:], xT[s])
                k_sb = kp.tile([_CIN, _KH * _KW, _F], f32r)
                nc.sync.dma_start(k_sb[:], kT[s])
                for fh in range(_NFH):
                    psums = [
                        ps.tile([_FH, _CHUNK], f32, name=f"psum_s{s}f{fh}c{c}", tag="psum")
                        for c in range(_NCH)
                    ]
                    for tap in range(_KH * _KW):
                        kh, kw = divmod(tap, _KW)
                        lhsT = k_sb[:, tap, fh * _FH:(fh + 1) * _FH]
                        for c in range(_NCH):
                            r0 = c * _ROWS + kh
                            rhs = x_sb[:, r0:r0 + _ROWS, kw:kw + _W]
                            nc.tensor.matmul(
                                psums[c][:], lhsT, rhs,
                                start=(tap == 0), stop=(tap == _KH * _KW - 1),
                            )
                    for c in range(_NCH):
                        o_sb = op.tile([_FH, _CHUNK], f32)
                        col = s * _NFH + fh
                        nc.scalar.activation(
                            o_sb[:], psums[c][:], ident,
                            bias=b_sb[:, col:col + 1], scale=1.0,
                        )
                        nc.sync.dma_start(
                            yT[s, fh, :, c * _CHUNK:(c + 1) * _CHUNK], o_sb[:]
                        )
    nc.compile()
    return nc


def get_nc():
    global _nc
    if _nc is None:
        _nc = _build_nc()
    return _nc


def _prep_inputs(x, classes, kernel, bias):
    cls = np.asarray(classes)[:, 0]
    k_per = np.asarray(kernel)[cls]          # [B, KH, KW, CIN, F]
    b_per = np.asarray(bias)[cls]            # [B, F]

    xpad = np.zeros((_B, _HP, _WP, _CIN), np.float32)
    xpad[:, 1:_H + 1, 1:_W + 1, :] = x
    xT_all = np.ascontiguousarray(xpad.transpose(0, 3, 1, 2))  # [B, CIN, HP, WP]
    kT_all = np.ascontiguousarray(
        k_per.reshape(_B, _KH * _KW, _CIN, _F).transpose(0, 2, 1, 3)
    )                                        # [B, CIN, 9, F]

    in_maps = []
    for i in range(_NCORES):
        lo = i * _BPC
        b_core = np.ascontiguousarray(
            b_per[lo:lo + _BPC].reshape(_BPC, _NFH, _FH).transpose(2, 0, 1)
        ).reshape(_FH, _BPC * _NFH)
        in_maps.append({
            "xT": np.ascontiguousarray(xT_all[lo:lo + _BPC]),
            "kT": np.ascontiguousarray(kT_all[lo:lo + _BPC]),
            "bT": b_core,
        })
    return in_maps


def _unshard(results):
    outs = []
    for r in results:
        yT = r["yT"]                          # [BPC, 2, 128, 4096]
        y = yT.reshape(_BPC, _F, _SP).transpose(0, 2, 1)
        outs.append(y.reshape(_BPC, _H, _W, _F))
    return np.ascontiguousarray(np.concatenate(outs, axis=0))


def run(x, classes, kernel, bias, trace=False):
    """Returns (y, BassKernelResults)."""
    from concourse.bass_utils import run_bass_kernel_spmd

    nc = get_nc()
    in_maps = _prep_inputs(x, classes, kernel, bias)
    res = run_bass_kernel_spmd(nc, in_maps, core_ids=list(range(_NCORES)), trace=trace)
    return _unshard(res.results), res


def kernel(x, classes, kernel, bias):
    y, _ = run(x, classes, kernel, bias)
    return y


# revision 16
# speedup vs baseline: 1.0605x; 1.0605x over previous
"""Conditional (class-routed) 3x3 SAME conv, data-parallel over batch on 8 TRN2 cores.

Strategy:
  - Host: gather per-sample expert kernel/bias (kernel[classes], bias[classes]),
    zero-pad x to 66x66 and transpose to channel-major [CIN, HP, WP]; shard
    batch 4 samples/core.
  - Device (per core): for each sample, conv = sum over 9 taps of a
    [CIN=128, F_half=128]^T @ [CIN=128, spatial-window 512] matmul accumulated
    in PSUM (fp32r = full-rate fp32 matmul). Bias added during PSUM->SBUF
    eviction, alternating Scalar/Vector engines.
  - x is loaded as 3 overlapping row-band tiles and the expert kernel as 2
    per-f-half tiles so sample-0 matmuls start after ~1.2MB of DMA instead of
    waiting for the whole 3.4MB shard.
  - Host: un-transpose [F, HW] -> [H, W, F] and concatenate shards.
"""

import numpy as np

_B, _H, _W, _CIN = 32, 64, 64, 128
_F, _KH, _KW = 256, 3, 3
_NCORES = 8
_BPC = _B // _NCORES          # 4 samples per core
_HP, _WP = _H + 2, _W + 2     # 66, 66 (zero-padded)
_SP = _H * _W                 # 4096 output positions
_FH = 128                     # output-channel half (PSUM partition dim)
_NFH = _F // _FH              # 2
_CHUNK = 512                  # spatial positions per PSUM bank
_NCH = _SP // _CHUNK          # 8
_ROWS = _CHUNK // _W          # 8 output rows per chunk
_NTAP = _KH * _KW             # 9

# overlapping x row-bands: band b serves chunks needing padded rows r0..r1-1
_BANDS = ((0, 18), (16, 42), (40, 66))
_CHUNK_BAND = (0, 0, 1, 1, 1, 2, 2, 2)

_nc = None


def _build_nc():
    import concourse.bacc as bacc
    import concourse.mybir as mybir
    import concourse.tile as tile
    from concourse.tile_rust import add_dep_helper

    f32 = mybir.dt.float32
    f32r = mybir.dt.float32r
    ident = mybir.ActivationFunctionType.Identity

    nc = bacc.Bacc("TRN2", target_bir_lowering=False, debug=False)
    xT = nc.dram_tensor("xT", (_BPC, _CIN, _HP, _WP), f32r, kind="ExternalInput")
    # kT[s, fh] is one f-half of the expert kernel: [CIN, NTAP, FH]
    kT = nc.dram_tensor("kT", (_BPC, _NFH, _CIN, _NTAP, _FH), f32r,
                        kind="ExternalInput")
    bT = nc.dram_tensor("bT", (_FH, _BPC * _NFH), f32, kind="ExternalInput")
    yT = nc.dram_tensor("yT", (_BPC, _NFH, _FH, _SP), f32, kind="ExternalOutput")

    with tile.TileContext(nc) as tc:
        with (
            tc.tile_pool(name="xp", bufs=6) as xp,
            tc.tile_pool(name="kp", bufs=4) as kp,
            tc.tile_pool(name="bp", bufs=1) as bp,
            tc.tile_pool(name="op", bufs=4) as op,
            tc.tile_pool(name="ps", bufs=8, space="PSUM") as ps,
        ):
            b_sb = bp.tile([_FH, _BPC * _NFH], f32)
            nc.sync.dma_start(b_sb[:], bT[:])
            gate_prev = None
            for s in range(_BPC):
                k_sb = []
                x_sb = []
                dmas = []

                def load_k(fh, s=s, k_sb=k_sb, dmas=dmas):
                    t = kp.tile([_CIN, _NTAP, _FH], f32r,
                                name=f"k_s{s}f{fh}", tag="k")
                    dmas.append(nc.sync.dma_start(t[:], kT[s, fh]))
                    k_sb.append(t)

                def load_band(b, s=s, x_sb=x_sb, dmas=dmas):
                    r0, r1 = _BANDS[b]
                    t = xp.tile([_CIN, r1 - r0, _WP], f32r,
                                name=f"x_s{s}b{b}", tag=f"xb{b}")
                    dmas.append(nc.sync.dma_start(t[:], xT[s, :, r0:r1, :]))
                    x_sb.append(t)

                # gate the first matmuls on the smallest possible prefix
                load_k(0)
                load_band(0)
                load_band(1)
                load_band(2)
                load_k(1)
                if gate_prev is not None:
                    # prefetch of sample s must not compete for HBM bandwidth
                    # with sample s-1's (still critical) input transfers
                    for d in dmas:
                        add_dep_helper(d.ins, gate_prev,
                                       reason="prefetch gated on prev sample")
                else:
                    # sample 0: keep band2/k1 off the wire until compute has
                    # started so k0/band0/band1 get the full HBM bandwidth
                    late = [dmas[3], dmas[4]]

                gate_this = None
                for fh in range(_NFH):
                    psums = [
                        ps.tile([_FH, _CHUNK], f32, name=f"psum_s{s}f{fh}c{c}",
                                tag="psum")
                        for c in range(_NCH)
                    ]
                    for tap in range(_NTAP):
                        kh, kw = divmod(tap, _KW)
                        lhsT = k_sb[fh][:, tap, :]
                        for c in range(_NCH):
                            b = _CHUNK_BAND[c]
                            r0 = c * _ROWS + kh - _BANDS[b][0]
                            rhs = x_sb[b][:, r0:r0 + _ROWS, kw:kw + _W]
                            mm = nc.tensor.matmul(
                                psums[c][:], lhsT, rhs,
                                start=(tap == 0), stop=(tap == _NTAP - 1),
                            )
                            if fh == 0 and tap == 0 and c == _NCH - 1:
                                gate_this = mm.ins
                            if (gate_prev is None and fh == 0 and tap == 0
                                    and c == 0):
                                for d in late:
                                    add_dep_helper(
                                        d.ins, mm.ins,
                                        reason="s0 late inputs after first MM")
                    col = s * _NFH + fh
                    bias_ap = b_sb[:, col:col + 1]
                    for p in range(_NCH // 2):
                        o_sb = op.tile([_FH, 2 * _CHUNK], f32,
                                       name=f"o_s{s}f{fh}p{p}", tag="o")
                        nc.vector.tensor_scalar_add(
                            o_sb[:, :_CHUNK], psums[2 * p][:], bias_ap)
                        nc.scalar.activation(
                            o_sb[:, _CHUNK:], psums[2 * p + 1][:], ident,
                            bias=bias_ap, scale=1.0,
                        )
                        nc.sync.dma_start(
                            yT[s, fh, :, 2 * p * _CHUNK:(2 * p + 2) * _CHUNK],
                            o_sb[:],
                        )
                gate_prev = gate_this
    nc.compile()
    return nc


def get_nc():
    global _nc
    if _nc is None:
        _nc = _build_nc()
    return _nc


def _prep_inputs(x, classes, kernel, bias):
    cls = np.asarray(classes)[:, 0]
    k_per = np.asarray(kernel)[cls]          # [B, KH, KW, CIN, F]
    b_per = np.asarray(bias)[cls]            # [B, F]

    xpad = np.zeros((_B, _HP, _WP, _CIN), np.float32)
    xpad[:, 1:_H + 1, 1:_W + 1, :] = x
    xT_all = np.ascontiguousarray(xpad.transpose(0, 3, 1, 2))  # [B, CIN, HP, WP]
    # [B, NFH, CIN, NTAP, FH]
    kT_all = np.ascontiguousarray(
        k_per.reshape(_B, _NTAP, _CIN, _NFH, _FH).transpose(0, 3, 2, 1, 4)
    )

    in_maps = []
    for i in range(_NCORES):
        lo = i * _BPC
        b_core = np.ascontiguousarray(
            b_per[lo:lo + _BPC].reshape(_BPC, _NFH, _FH).transpose(2, 0, 1)
        ).reshape(_FH, _BPC * _NFH)
        in_maps.append({
            "xT": np.ascontiguousarray(xT_all[lo:lo + _BPC]),
            "kT": np.ascontiguousarray(kT_all[lo:lo + _BPC]),
            "bT": b_core,
        })
    return in_maps


def _unshard(results):
    outs = []
    for r in results:
        yT = r["yT"]                          # [BPC, 2, 128, 4096]
        y = yT.reshape(_BPC, _F, _SP).transpose(0, 2, 1)
        outs.append(y.reshape(_BPC, _H, _W, _F))
    return np.ascontiguousarray(np.concatenate(outs, axis=0))


def run(x, classes, kernel, bias, trace=False):
    """Returns (y, BassKernelResults)."""
    from concourse.bass_utils import run_bass_kernel_spmd

    nc = get_nc()
    in_maps = _prep_inputs(x, classes, kernel, bias)
    res = run_bass_kernel_spmd(nc, in_maps, core_ids=list(range(_NCORES)), trace=trace)
    return _unshard(res.results), res


def kernel(x, classes, kernel, bias):
    y, _ = run(x, classes, kernel, bias)
    return y


# revision 17
# speedup vs baseline: 1.1559x; 1.0900x over previous
"""Conditional (class-routed) 3x3 SAME conv, data-parallel over batch on 8 TRN2 cores.

Strategy:
  - Host: gather per-sample expert kernel/bias (kernel[classes], bias[classes]),
    zero-pad x to 66x66 and transpose to channel-major [CIN, HP, WP]; shard
    batch 4 samples/core.
  - Device (per core): for each sample, conv = sum over 9 taps of a
    [CIN=128, F_half=128]^T @ [CIN=128, spatial-window 512] matmul accumulated
    in PSUM (fp16 operands, fp32 PSUM accumulation). Bias added during PSUM->SBUF
    eviction, alternating Scalar/Vector engines.
  - x is loaded as 3 overlapping row-band tiles and the expert kernel as 2
    per-f-half tiles so sample-0 matmuls start after ~1.2MB of DMA instead of
    waiting for the whole 3.4MB shard.
  - Host: un-transpose [F, HW] -> [H, W, F] and concatenate shards.
"""

import numpy as np

_B, _H, _W, _CIN = 32, 64, 64, 128
_F, _KH, _KW = 256, 3, 3
_NCORES = 8
_BPC = _B // _NCORES          # 4 samples per core
_HP, _WP = _H + 2, _W + 2     # 66, 66 (zero-padded)
_SP = _H * _W                 # 4096 output positions
_FH = 128                     # output-channel half (PSUM partition dim)
_NFH = _F // _FH              # 2
_CHUNK = 512                  # spatial positions per PSUM bank
_NCH = _SP // _CHUNK          # 8
_ROWS = _CHUNK // _W          # 8 output rows per chunk
_NTAP = _KH * _KW             # 9

# overlapping x row-bands: band b serves chunks needing padded rows r0..r1-1
_BANDS = ((0, 18), (16, 42), (40, 66))
_CHUNK_BAND = (0, 0, 1, 1, 1, 2, 2, 2)

_nc = None


def _build_nc():
    import concourse.bacc as bacc
    import concourse.mybir as mybir
    import concourse.tile as tile
    from concourse.tile_rust import add_dep_helper

    f32 = mybir.dt.float32
    f16 = mybir.dt.float16
    ident = mybir.ActivationFunctionType.Identity

    nc = bacc.Bacc("TRN2", target_bir_lowering=False, debug=False)
    xT = nc.dram_tensor("xT", (_BPC, _CIN, _HP, _WP), f16, kind="ExternalInput")
    # kT[s, fh] is one f-half of the expert kernel: [CIN, NTAP, FH]
    kT = nc.dram_tensor("kT", (_BPC, _NFH, _CIN, _NTAP, _FH), f16,
                        kind="ExternalInput")
    bT = nc.dram_tensor("bT", (_FH, _BPC * _NFH), f32, kind="ExternalInput")
    yT = nc.dram_tensor("yT", (_BPC, _NFH, _FH, _SP), f32, kind="ExternalOutput")

    with tile.TileContext(nc) as tc:
        with (
            tc.tile_pool(name="xp", bufs=6) as xp,
            tc.tile_pool(name="kp", bufs=4) as kp,
            tc.tile_pool(name="bp", bufs=1) as bp,
            tc.tile_pool(name="op", bufs=4) as op,
            tc.tile_pool(name="ps", bufs=8, space="PSUM") as ps,
        ):
            b_sb = bp.tile([_FH, _BPC * _NFH], f32)
            nc.sync.dma_start(b_sb[:], bT[:])
            gate_prev = None
            for s in range(_BPC):
                k_sb = []
                x_sb = []
                dmas = []

                def load_k(fh, s=s, k_sb=k_sb, dmas=dmas):
                    t = kp.tile([_CIN, _NTAP, _FH], f16,
                                name=f"k_s{s}f{fh}", tag="k")
                    dmas.append(nc.sync.dma_start(t[:], kT[s, fh]))
                    k_sb.append(t)

                def load_band(b, s=s, x_sb=x_sb, dmas=dmas):
                    r0, r1 = _BANDS[b]
                    t = xp.tile([_CIN, r1 - r0, _WP], f16,
                                name=f"x_s{s}b{b}", tag=f"xb{b}")
                    dmas.append(nc.sync.dma_start(t[:], xT[s, :, r0:r1, :]))
                    x_sb.append(t)

                # gate the first matmuls on the smallest possible prefix
                load_k(0)
                load_band(0)
                load_band(1)
                load_band(2)
                load_k(1)
                if gate_prev is not None:
                    # prefetch of sample s must not compete for HBM bandwidth
                    # with sample s-1's (still critical) input transfers
                    for d in dmas:
                        add_dep_helper(d.ins, gate_prev,
                                       reason="prefetch gated on prev sample")
                else:
                    # sample 0: keep band2/k1 off the wire until compute has
                    # started so k0/band0/band1 get the full HBM bandwidth
                    late = [dmas[3], dmas[4]]

                gate_this = None
                for fh in range(_NFH):
                    psums = [
                        ps.tile([_FH, _CHUNK], f32, name=f"psum_s{s}f{fh}c{c}",
                                tag="psum")
                        for c in range(_NCH)
                    ]
                    for tap in range(_NTAP):
                        kh, kw = divmod(tap, _KW)
                        lhsT = k_sb[fh][:, tap, :]
                        for c in range(_NCH):
                            b = _CHUNK_BAND[c]
                            r0 = c * _ROWS + kh - _BANDS[b][0]
                            rhs = x_sb[b][:, r0:r0 + _ROWS, kw:kw + _W]
                            mm = nc.tensor.matmul(
                                psums[c][:], lhsT, rhs,
                                start=(tap == 0), stop=(tap == _NTAP - 1),
                            )
                            if fh == 0 and tap == 0 and c == _NCH - 1:
                                gate_this = mm.ins
                            if (gate_prev is None and fh == 0 and tap == 0
                                    and c == 0):
                                for d in late:
                                    add_dep_helper(
                                        d.ins, mm.ins,
                                        reason="s0 late inputs after first MM")
                    col = s * _NFH + fh
                    bias_ap = b_sb[:, col:col + 1]
                    for p in range(_NCH // 2):
                        o_sb = op.tile([_FH, 2 * _CHUNK], f32,
                                       name=f"o_s{s}f{fh}p{p}", tag="o")
                        nc.vector.tensor_scalar_add(
                            o_sb[:, :_CHUNK], psums[2 * p][:], bias_ap)
                        nc.scalar.activation(
                            o_sb[:, _CHUNK:], psums[2 * p + 1][:], ident,
                            bias=bias_ap, scale=1.0,
                        )
                        nc.sync.dma_start(
                            yT[s, fh, :, 2 * p * _CHUNK:(2 * p + 2) * _CHUNK],
                            o_sb[:],
                        )
                gate_prev = gate_this
    nc.compile()
    return nc


def get_nc():
    global _nc
    if _nc is None:
        _nc = _build_nc()
    return _nc


def _prep_inputs(x, classes, kernel, bias):
    cls = np.asarray(classes)[:, 0]
    k_per = np.asarray(kernel)[cls]          # [B, KH, KW, CIN, F]
    b_per = np.asarray(bias)[cls]            # [B, F]

    xpad = np.zeros((_B, _HP, _WP, _CIN), np.float16)
    xpad[:, 1:_H + 1, 1:_W + 1, :] = np.asarray(x).astype(np.float16)
    xT_all = np.ascontiguousarray(xpad.transpose(0, 3, 1, 2))  # [B, CIN, HP, WP]
    # [B, NFH, CIN, NTAP, FH]
    kT_all = np.ascontiguousarray(
        k_per.reshape(_B, _NTAP, _CIN, _NFH, _FH).transpose(0, 3, 2, 1, 4)
    ).astype(np.float16)

    in_maps = []
    for i in range(_NCORES):
        lo = i * _BPC
        b_core = np.ascontiguousarray(
            b_per[lo:lo + _BPC].reshape(_BPC, _NFH, _FH).transpose(2, 0, 1)
        ).reshape(_FH, _BPC * _NFH)
        in_maps.append({
            "xT": np.ascontiguousarray(xT_all[lo:lo + _BPC]),
            "kT": np.ascontiguousarray(kT_all[lo:lo + _BPC]),
            "bT": b_core,
        })
    return in_maps


def _unshard(results):
    outs = []
    for r in results:
        yT = r["yT"]                          # [BPC, 2, 128, 4096]
        y = yT.reshape(_BPC, _F, _SP).transpose(0, 2, 1)
        outs.append(y.reshape(_BPC, _H, _W, _F))
    return np.ascontiguousarray(np.concatenate(outs, axis=0))


def run(x, classes, kernel, bias, trace=False):
    """Returns (y, BassKernelResults)."""
    from concourse.bass_utils import run_bass_kernel_spmd

    nc = get_nc()
    in_maps = _prep_inputs(x, classes, kernel, bias)
    res = run_bass_kernel_spmd(nc, in_maps, core_ids=list(range(_NCORES)), trace=trace)
    return _unshard(res.results), res


def kernel(x, classes, kernel, bias):
    y, _ = run(x, classes, kernel, bias)
    return y


# revision 18
# speedup vs baseline: 1.1672x; 1.0098x over previous
"""Conditional (class-routed) 3x3 SAME conv, data-parallel over batch on 8 TRN2 cores.

Strategy:
  - Host: gather per-sample expert kernel/bias (kernel[classes], bias[classes]),
    zero-pad x to 66x66 and transpose to channel-major [CIN, HP, WP]; shard
    batch 4 samples/core.
  - Device (per core): for each sample, conv = sum over 9 taps of a
    [CIN=128, F_half=128]^T @ [CIN=128, spatial-window 512] matmul accumulated
    in PSUM (fp16 operands, fp32 PSUM accumulation). Bias added during PSUM->SBUF
    eviction, alternating Scalar/Vector engines.
  - x is loaded as 3 overlapping row-band tiles and the expert kernel as 2
    per-f-half tiles so sample-0 matmuls start after ~1.2MB of DMA instead of
    waiting for the whole 3.4MB shard.
  - Host: un-transpose [F, HW] -> [H, W, F] and concatenate shards.
"""

import numpy as np

_B, _H, _W, _CIN = 32, 64, 64, 128
_F, _KH, _KW = 256, 3, 3
_NCORES = 8
_BPC = _B // _NCORES          # 4 samples per core
_HP, _WP = _H + 2, _W + 2     # 66, 66 (zero-padded)
_SP = _H * _W                 # 4096 output positions
_FH = 128                     # output-channel half (PSUM partition dim)
_NFH = _F // _FH              # 2
_CHUNK = 512                  # spatial positions per PSUM bank
_NCH = _SP // _CHUNK          # 8
_ROWS = _CHUNK // _W          # 8 output rows per chunk
_NTAP = _KH * _KW             # 9

# overlapping x row-bands: band b serves chunks needing padded rows r0..r1-1
_BANDS = ((0, 18), (16, 42), (40, 66))
_CHUNK_BAND = (0, 0, 1, 1, 1, 2, 2, 2)

_nc = None


def _build_nc():
    import concourse.bacc as bacc
    import concourse.mybir as mybir
    import concourse.tile as tile
    from concourse.tile_rust import add_dep_helper

    f32 = mybir.dt.float32
    f16 = mybir.dt.float16
    ident = mybir.ActivationFunctionType.Identity

    nc = bacc.Bacc("TRN2", target_bir_lowering=False, debug=False)
    xT = nc.dram_tensor("xT", (_BPC, _CIN, _HP, _WP), f16, kind="ExternalInput")
    # kT[s, fh] is one f-half of the expert kernel: [CIN, NTAP, FH]
    kT = nc.dram_tensor("kT", (_BPC, _NFH, _CIN, _NTAP, _FH), f16,
                        kind="ExternalInput")
    bT = nc.dram_tensor("bT", (_FH, _BPC * _NFH), f32, kind="ExternalInput")
    yT = nc.dram_tensor("yT", (_BPC, _NFH, _FH, _SP), f16, kind="ExternalOutput")

    with tile.TileContext(nc) as tc:
        with (
            tc.tile_pool(name="xp", bufs=6) as xp,
            tc.tile_pool(name="kp", bufs=4) as kp,
            tc.tile_pool(name="bp", bufs=1) as bp,
            tc.tile_pool(name="op", bufs=4) as op,
            tc.tile_pool(name="ps", bufs=8, space="PSUM") as ps,
        ):
            b_sb = bp.tile([_FH, _BPC * _NFH], f32)
            nc.sync.dma_start(b_sb[:], bT[:])
            gate_prev = None
            for s in range(_BPC):
                k_sb = []
                x_sb = []
                dmas = []

                def load_k(fh, s=s, k_sb=k_sb, dmas=dmas):
                    t = kp.tile([_CIN, _NTAP, _FH], f16,
                                name=f"k_s{s}f{fh}", tag="k")
                    dmas.append(nc.sync.dma_start(t[:], kT[s, fh]))
                    k_sb.append(t)

                def load_band(b, s=s, x_sb=x_sb, dmas=dmas):
                    r0, r1 = _BANDS[b]
                    t = xp.tile([_CIN, r1 - r0, _WP], f16,
                                name=f"x_s{s}b{b}", tag=f"xb{b}")
                    dmas.append(nc.sync.dma_start(t[:], xT[s, :, r0:r1, :]))
                    x_sb.append(t)

                # gate the first matmuls on the smallest possible prefix
                load_k(0)
                load_band(0)
                load_band(1)
                load_band(2)
                load_k(1)
                if gate_prev is not None:
                    # prefetch of sample s must not compete for HBM bandwidth
                    # with sample s-1's (still critical) input transfers
                    for d in dmas:
                        add_dep_helper(d.ins, gate_prev,
                                       reason="prefetch gated on prev sample")
                else:
                    # sample 0: keep band2/k1 off the wire until compute has
                    # started so k0/band0/band1 get the full HBM bandwidth
                    late = [dmas[3], dmas[4]]

                gate_this = None
                for fh in range(_NFH):
                    psums = [
                        ps.tile([_FH, _CHUNK], f32, name=f"psum_s{s}f{fh}c{c}",
                                tag="psum")
                        for c in range(_NCH)
                    ]
                    for tap in range(_NTAP):
                        kh, kw = divmod(tap, _KW)
                        lhsT = k_sb[fh][:, tap, :]
                        for c in range(_NCH):
                            b = _CHUNK_BAND[c]
                            r0 = c * _ROWS + kh - _BANDS[b][0]
                            rhs = x_sb[b][:, r0:r0 + _ROWS, kw:kw + _W]
                            mm = nc.tensor.matmul(
                                psums[c][:], lhsT, rhs,
                                start=(tap == 0), stop=(tap == _NTAP - 1),
                            )
                            if fh == 0 and tap == 0 and c == _NCH - 1:
                                gate_this = mm.ins
                            if (gate_prev is None and fh == 0 and tap == 0
                                    and c == 0):
                                for d in late:
                                    add_dep_helper(
                                        d.ins, mm.ins,
                                        reason="s0 late inputs after first MM")
                    col = s * _NFH + fh
                    bias_ap = b_sb[:, col:col + 1]
                    for p in range(_NCH // 2):
                        o_sb = op.tile([_FH, 2 * _CHUNK], f16,
                                       name=f"o_s{s}f{fh}p{p}", tag="o")
                        nc.vector.tensor_scalar_add(
                            o_sb[:, :_CHUNK], psums[2 * p][:], bias_ap)
                        nc.scalar.activation(
                            o_sb[:, _CHUNK:], psums[2 * p + 1][:], ident,
                            bias=bias_ap, scale=1.0,
                        )
                        nc.sync.dma_start(
                            yT[s, fh, :, 2 * p * _CHUNK:(2 * p + 2) * _CHUNK],
                            o_sb[:],
                        )
                gate_prev = gate_this
    nc.compile()
    return nc


def get_nc():
    global _nc
    if _nc is None:
        _nc = _build_nc()
    return _nc


def _prep_inputs(x, classes, kernel, bias):
    cls = np.asarray(classes)[:, 0]
    k_per = np.asarray(kernel)[cls]          # [B, KH, KW, CIN, F]
    b_per = np.asarray(bias)[cls]            # [B, F]

    xpad = np.zeros((_B, _HP, _WP, _CIN), np.float16)
    xpad[:, 1:_H + 1, 1:_W + 1, :] = np.asarray(x).astype(np.float16)
    xT_all = np.ascontiguousarray(xpad.transpose(0, 3, 1, 2))  # [B, CIN, HP, WP]
    # [B, NFH, CIN, NTAP, FH]
    kT_all = np.ascontiguousarray(
        k_per.reshape(_B, _NTAP, _CIN, _NFH, _FH).transpose(0, 3, 2, 1, 4)
    ).astype(np.float16)

    in_maps = []
    for i in range(_NCORES):
        lo = i * _BPC
        b_core = np.ascontiguousarray(
            b_per[lo:lo + _BPC].reshape(_BPC, _NFH, _FH).transpose(2, 0, 1)
        ).reshape(_FH, _BPC * _NFH)
        in_maps.append({
            "xT": np.ascontiguousarray(xT_all[lo:lo + _BPC]),
            "kT": np.ascontiguousarray(kT_all[lo:lo + _BPC]),
            "bT": b_core,
        })
    return in_maps


def _unshard(results):
    outs = []
    for r in results:
        yT = r["yT"].astype(np.float32)       # [BPC, 2, 128, 4096]
        y = yT.reshape(_BPC, _F, _SP).transpose(0, 2, 1)
        outs.append(y.reshape(_BPC, _H, _W, _F))
    return np.ascontiguousarray(np.concatenate(outs, axis=0))


def run(x, classes, kernel, bias, trace=False):
    """Returns (y, BassKernelResults)."""
    from concourse.bass_utils import run_bass_kernel_spmd

    nc = get_nc()
    in_maps = _prep_inputs(x, classes, kernel, bias)
    res = run_bass_kernel_spmd(nc, in_maps, core_ids=list(range(_NCORES)), trace=trace)
    return _unshard(res.results), res


def kernel(x, classes, kernel, bias):
    y, _ = run(x, classes, kernel, bias)
    return y
